# revision 9
# baseline (speedup 1.0000x reference)
"""Adaptive average pooling (8,384,384,64) NHWC -> (8,7,7,64) on 8 TRN2 NeuronCores.

Pure data parallel: one batch sample per core, no collectives. Per core:
  - Stream the sample as 21 slabs (3 H-chunks x 7 adaptive W-windows), each
    (128 h x 56 w x 64 c) = 1.79 MB, through an 11-slab SBUF ring. DMAs
    alternate between the two HWDGE rings (sync/SP and scalar/ACT engines).
  - DVE tensor_reduce sums each slab over the (strided) w axis ->
    t[h, j, c] (128 x 448 per H-chunk).
  - TensorE accumulates the three H-chunks against the H-window weight
    matrix P (128x7 per chunk) in PSUM: y[i,(j,c)] = sum_h P[h,i]*t[h,(j,c)].
  - One DVE multiply applies the 1/w_window scaling; DMA out (7 x 448).

Raw Bass blocks with explicit semaphores (TileContext's generated sync
exceeds this toolchain's per-instruction sync-wait limits). Memory-bound:
~37.75 MB HBM traffic per core vs ~54 us of DVE and ~4 us of PE work.
"""

import numpy as np

import concourse.bass as bass
import concourse.mybir as mybir
from concourse.bass_utils import run_bass_kernel_spmd

B, H, W, C = 8, 384, 384, 64
OUT = 7
N_CORES = 8
KH = H // 128  # 3 H-chunks of 128 rows
WMAX = 56  # largest adaptive window along W
NSLAB = KH * OUT  # 21 slabs per core
SLAB = WMAX * C  # 3584 f32 per partition line
RING = 11  # SBUF ring depth in slabs

_F32 = mybir.dt.float32


def _windows(d, out):
    starts = np.floor(np.arange(out) * d / out).astype(np.int64)
    ends = np.ceil((np.arange(out) + 1) * d / out).astype(np.int64)
    return starts, ends - starts


def _build():
    nc = bass.Bass()
    x = nc.declare_dram_parameter("x", [H, W * C], _F32, isOutput=False)
    pmat = nc.declare_dram_parameter("pmat", [128, KH * OUT], _F32, isOutput=False)
    invw = nc.declare_dram_parameter("invw", [OUT, OUT * C], _F32, isOutput=False)
    out = nc.declare_dram_parameter("out", [OUT, OUT * C], _F32, isOutput=True)

    ws, wsz = _windows(W, OUT)
    # Read a full WMAX-wide slab for every window (uniform shape); the reduce
    # AP selects the exact window inside it. s_read/off handle the last
    # window, whose 56-wide read would run off the right edge.
    reads = []
    for j in range(OUT):
        s, sz = int(ws[j]), int(wsz[j])
        s_read = min(s, W - WMAX)
        reads.append((s_read, s - s_read, sz))

    with (
        nc.sbuf_tensor([128, RING * SLAB], _F32) as xring,
        nc.sbuf_tensor([128, KH * OUT * C], _F32) as t3,
        nc.sbuf_tensor([128, KH * OUT], _F32) as p_sb,
        nc.sbuf_tensor([OUT, OUT * C], _F32) as invw_sb,
        nc.sbuf_tensor([OUT, OUT * C], _F32) as y_sb,
        nc.psum_tensor([128, OUT * C], _F32) as psum,
        nc.semaphore("dve_sem") as dve_sem,
        nc.semaphore("const_sem") as const_sem,
        nc.semaphore("pe_sem") as pe_sem,
        nc.semaphore("mul_sem") as mul_sem,
        nc.semaphore("out_sem") as out_sem,
    ):
        slab_sems = [nc.alloc_semaphore(f"slab{r}") for r in range(RING)]

        def issue_dmas(eng, parity):
            for n in range(NSLAB):
                if n % 2 != parity:
                    continue
                k, j = divmod(n, OUT)
                s_read, _, _ = reads[j]
                r = n % RING
                if n >= RING:
                    # WAR: the slab's previous occupant must have been reduced
                    eng.wait_ge(dve_sem, n - RING + 1)
                eng.dma_start(
                    out=xring[:, r * SLAB : (r + 1) * SLAB],
                    in_=x[k * 128 : (k + 1) * 128, s_read * C : (s_read + WMAX) * C],
                ).then_inc(slab_sems[r], 16)

        with nc.Block() as block:

            @block.sync
            def _(sync):
                issue_dmas(sync, 0)
                sync.wait_ge(mul_sem, 1)
                sync.dma_start(out=out[:], in_=y_sb[:]).then_inc(out_sem, 16)
                sync.wait_ge(out_sem, 16)

            @block.scalar
            def _(scalar):
                issue_dmas(scalar, 1)

            @block.gpsimd
            def _(gpsimd):
                gpsimd.dma_start(out=p_sb[:], in_=pmat[:]).then_inc(const_sem, 16)
                gpsimd.dma_start(out=invw_sb[:], in_=invw[:]).then_inc(const_sem, 16)

            @block.vector
            def _(vector):
                for n in range(NSLAB):
                    k, j = divmod(n, OUT)
                    _, off, sz = reads[j]
                    r = n % RING
                    vector.wait_ge(slab_sems[r], 16 * (n // RING + 1))
                    base = r * SLAB + off * C
                    vector.tensor_reduce(
                        out=t3[:, (k * OUT + j) * C : (k * OUT + j + 1) * C],
                        in_=xring[:, base : base + sz * C].rearrange(
                            "p (w c) -> p c w", c=C
                        ),
                        axis=mybir.AxisListType.X,
                        op=mybir.AluOpType.add,
                    ).then_inc(dve_sem, 1)
                vector.wait_ge(pe_sem, 1)
                vector.wait_ge(const_sem, 32)
                vector.tensor_mul(y_sb[:], psum[:OUT, :], invw_sb[:]).then_inc(
                    mul_sem, 1
                )

            @block.tensor
            def _(tensor):
                tensor.wait_ge(const_sem, 32)
                for k in range(KH):
                    tensor.wait_ge(dve_sem, OUT * (k + 1))
                    mm = tensor.matmul(
                        psum[:OUT, :],
                        p_sb[:, k * OUT : (k + 1) * OUT],
                        t3[:, k * OUT * C : (k + 1) * OUT * C],
                        start=(k == 0),
                        stop=(k == KH - 1),
                    )
                mm.then_inc(pe_sem, 1)

    return nc


def _consts():
    hs, hsz = _windows(H, OUT)
    p = np.zeros((128, KH * OUT), np.float32)
    for i in range(OUT):
        for h in range(int(hs[i]), int(hs[i] + hsz[i])):
            k, r = divmod(h, 128)
            p[r, k * OUT + i] = 1.0 / float(hsz[i])
    ws, wsz = _windows(W, OUT)
    inv = np.zeros((OUT, OUT * C), np.float32)
    for j in range(OUT):
        inv[:, j * C : (j + 1) * C] = 1.0 / float(wsz[j])
    return p, inv


_NC_CACHE = None


def _run(x, **kwargs):
    global _NC_CACHE
    if _NC_CACHE is None:
        _NC_CACHE = _build()
    nc = _NC_CACHE
    p, inv = _consts()
    x = np.ascontiguousarray(np.asarray(x, dtype=np.float32))
    in_maps = [
        {"x": x[b].reshape(H, W * C), "pmat": p, "invw": inv} for b in range(N_CORES)
    ]
    res = run_bass_kernel_spmd(nc, in_maps, core_ids=list(range(N_CORES)), **kwargs)
    y = np.stack(
        [res.results[b]["out"].reshape(OUT, OUT, C) for b in range(N_CORES)]
    )
    return y, res


def kernel(x: np.ndarray) -> np.ndarray:
    y, _ = _run(x)
    return y
